# revision 13
# baseline (speedup 1.0000x reference)
"""MultiHeadAttention Trainium2 kernel (8 NeuronCores).

Reference computation (B=4, T=2048, D=512, H=8, head_dim=64):
    q = split_heads(queries @ Wq + bq); k, v likewise
    wei = softmax(q k^T / sqrt(512) + (-1e9) * mask)   # mask: causal
    out = merge_heads(wei @ v) @ Wo + bo

Sharding: core = 2*b + g  (b in 0..3 batches, g in 0..1 groups of 4 heads).
Each core computes attention for its batch with its 4 heads plus a partial
output projection through its half of Wo's rows. Host sums the two partials
per batch and adds bo + bv @ Wo (the value bias is separable: softmax rows
sum to 1, so V += bv shifts each head's output by exactly bv).

Device-side layout: everything transposed (feature dim on partitions) so no
on-chip transposes are needed:
    Q^T, K^T : [64, T] per head, heads of a pair at partition offsets 0/64
    S^T blocks [128 k, q] via lhsT=K^T_j, rhs=Q^T; the pair's two K=64
        matmuls land on PE row groups 0/64
    P^T = exp(scale * S^T) on ScalarE (both heads in one ACTIVATE),
        causal handled by skipping dead ranges + one broadcast 0/1
        lower-triangular multiply on the diagonal 128x128 windows
    O^T[65, q] accumulates lhsT=[V | ones]; row 64 = softmax denominators,
        broadcast across partitions via a DRAM-bounce DMA (gpsimd queue),
        fast-approx reciprocal on VectorE
    Y[t, 512] partial = stacked normalized O^T pairs against Wo_g rows

Engine budget: ScalarE runs ONLY the softmax exps (the per-core floor,
~70 ns/KElem); all PSUM->SBUF copies are on VectorE (proj bias-add) and
GpSimd (V tiles, output tiles). The attention inner loop is software
pipelined (S_{j+1} issued before PV_j) so the in-order PE queue never
waits on the exp it just produced. Input DMAs are split into [128, 512]
pieces across three rings (sync/gpsimd/vector), priority-ordered so the
first S block fires ~7us in; out-projection chunks + output DMAs are
injected into the second attention pass to keep the tail short.
"""

import sys

if "/opt/trn_rl_repo" not in sys.path:
    sys.path.insert(0, "/opt/trn_rl_repo")

import numpy as np
import ml_dtypes

B, T, D, H = 4, 2048, 512, 8
HPG = 4                 # heads per group (per core)
HD = 64                 # head dim
DG = HPG * HD           # 256 feature dims per group
N_CORES = 8
SM_SCALE = float(D) ** -0.5   # module scales by full d_k = 512

_BF16 = ml_dtypes.bfloat16

_compiled = None


def _build():
    import concourse.bass as bass
    import concourse.bacc as bacc
    import concourse.tile as tile
    import concourse.mybir as mybir

    f32 = mybir.dt.float32
    bf16 = mybir.dt.bfloat16
    Exp = mybir.ActivationFunctionType.Exp
    add = mybir.AluOpType.add
    mult = mybir.AluOpType.mult

    nc = bacc.Bacc("TRN2", target_bir_lowering=False, debug=False,
                   num_devices=N_CORES)

    xq = nc.dram_tensor("xq_t", [D, T], bf16, kind="ExternalInput").ap()
    xk = nc.dram_tensor("xk_t", [D, T], bf16, kind="ExternalInput").ap()
    xv = nc.dram_tensor("xv_t", [D, T], bf16, kind="ExternalInput").ap()
    wq = nc.dram_tensor("wq", [D, DG], bf16, kind="ExternalInput").ap()
    wk = nc.dram_tensor("wk", [D, DG], bf16, kind="ExternalInput").ap()
    wv = nc.dram_tensor("wv", [D, DG], bf16, kind="ExternalInput").ap()
    wo = nc.dram_tensor("wo", [DG, D], bf16, kind="ExternalInput").ap()
    bq = nc.dram_tensor("bq", [1, DG], f32, kind="ExternalInput").ap()
    bk = nc.dram_tensor("bk", [1, DG], f32, kind="ExternalInput").ap()
    y = nc.dram_tensor("y", [T, D], bf16, kind="ExternalOutput").ap()

    NT = T // 512        # 4 t/q chunks of 512
    NB = T // 128        # 16 t/k blocks of 128
    ND = D // 128        # 4 contraction chunks over D

    with tile.TileContext(nc) as tc:
        with (
            tc.tile_pool(name="const", bufs=1) as const,
            tc.tile_pool(name="pt", bufs=10) as ppool,
            tc.tile_pool(name="rc", bufs=4) as rcpool,
            tc.tile_pool(name="ysb", bufs=6) as ypool,
            tc.tile_pool(name="psA", bufs=2, space="PSUM") as psA,
            tc.tile_pool(name="psO", bufs=2, space="PSUM") as psO,
            tc.tile_pool(name="dscratch", bufs=2, space="DRAM") as dpool,
        ):
            # ---- input DMAs: [128, 512] pieces, priority-ordered ---------
            bq_sb = const.tile([128, 2], f32, tag="bq")
            nc.sync.dma_start(out=bq_sb[:],
                              in_=bq.rearrange("o (c p) -> p (o c)", p=128))
            bk_sb = const.tile([128, 2], f32, tag="bk")
            nc.sync.dma_start(out=bk_sb[:],
                              in_=bk.rearrange("o (c p) -> p (o c)", p=128))

            wq_sb = const.tile([128, ND, DG], bf16, tag="wq")
            nc.sync.dma_start(
                out=wq_sb[:], in_=wq.rearrange("(c p) m -> p c m", p=128))
            wk_sb = const.tile([128, ND, DG], bf16, tag="wk")
            nc.sync.dma_start(
                out=wk_sb[:], in_=wk.rearrange("(c p) m -> p c m", p=128))

            def stage(name):
                return [const.tile([128, T], bf16, tag=f"{name}{dc}",
                                   name=f"{name}{dc}") for dc in range(ND)]

            xq_sb, xk_sb, xv_sb = stage("xq"), stage("xk"), stage("xv")

            def load_half(sb, dram, h):
                s = slice(1024 * h, 1024 * (h + 1))
                for dc in range(ND):
                    r = slice(128 * dc, 128 * (dc + 1))
                    nc.sync.dma_start(out=sb[dc][:, s], in_=dram[r, s])

            wv_sb = const.tile([128, ND, DG], bf16, tag="wv")
            wo_sb = const.tile([128, 2, D], bf16, tag="wo")

            # single HWDGE ring (sync), priority-ordered for the pipeline
            load_half(xq_sb, xq, 0)
            load_half(xk_sb, xk, 0)
            nc.sync.dma_start(
                out=wv_sb[:], in_=wv.rearrange("(c p) m -> p c m", p=128))
            load_half(xv_sb, xv, 0)
            load_half(xq_sb, xq, 1)
            load_half(xk_sb, xk, 1)
            load_half(xv_sb, xv, 1)
            nc.sync.dma_start(out=wo_sb[:],
                              in_=wo.rearrange("(c p) n -> p c n", p=128))

            # small PE warm-up while the first DMA pieces land.
            # NOTE every emission keeps an EVEN number of "ps"-tag psum
            # allocations so the attention S tiles stay double-buffered
            # (odd counts serialize the S/exp pipeline onto one buffer).
            warm = const.tile([128, 512], bf16, tag="warm")
            nc.vector.memset(warm[:], 0.0)
            for _ in range(2):
                wps = psA.tile([128, 1024], f32, tag="ps", name="wps")
                for _ in range(3):
                    nc.tensor.matmul(wps[:, :512], lhsT=warm[:, :128],
                                     rhs=warm[:], start=True, stop=True)

            # 0/1 lower-triangular (keep k <= q): tri[k, q] = (q - k >= 0)
            tri = const.tile([128, 128], bf16, tag="tri")
            nc.gpsimd.memset(tri[:], 1.0)
            nc.gpsimd.affine_select(
                out=tri[:], in_=tri[:],
                compare_op=mybir.AluOpType.is_ge, fill=0.0,
                base=0, pattern=[[1, 128]], channel_multiplier=-1)

            qT = const.tile([128, 2, T], bf16, tag="qT")
            kT = const.tile([128, 2, T], bf16, tag="kT")
            vA = const.tile([128, NB, HPG, HD + 1], bf16, tag="vA")
            nc.gpsimd.memset(vA[:, :, :, HD:HD + 1], 1.0)
            oTn = const.tile([128, 2, T], bf16, tag="oTn")

            def proj_qk(dst, w_sb, x_sb, b_sb, pc, c):
                # one 512-wide t-chunk of Q^T / K^T for head pair pc
                ps = psA.tile([128, 1024], f32, tag="ps", name="ps")[:, :512]
                for dc in range(ND):
                    nc.tensor.matmul(
                        ps,
                        lhsT=w_sb[:, dc, 128 * pc:128 * (pc + 1)],
                        rhs=x_sb[dc][:, 512 * c:512 * (c + 1)],
                        start=(dc == 0), stop=(dc == ND - 1))
                # bias-add + bf16 cast on VectorE (keeps ScalarE exp-only)
                nc.vector.tensor_scalar(
                    dst[:, pc, 512 * c:512 * (c + 1)], ps,
                    b_sb[:, pc:pc + 1], None, add)

            def proj_v(tb):
                ps = psA.tile([128, 1024], f32, tag="ps",
                              name="vps")[:, :DG]
                for dc in range(ND):
                    nc.tensor.matmul(
                        ps,
                        lhsT=xv_sb[dc][:, 128 * tb:128 * (tb + 1)],
                        rhs=wv_sb[:, dc, :],
                        start=(dc == 0), stop=(dc == ND - 1))
                nc.vector.tensor_copy(
                    vA[:, tb, :, 0:HD],
                    ps.rearrange("p (h d) -> p h d", h=HPG))

            def out_proj(tb):
                yp = psA.tile([128, 1024], f32, tag="ps",
                              name="yp")[:, :512]
                for pair in range(2):
                    nc.tensor.matmul(
                        yp,
                        lhsT=oTn[:, pair, 128 * tb:128 * (tb + 1)],
                        rhs=wo_sb[:, pair, :],
                        start=(pair == 0), stop=(pair == 1))
                ysb = ypool.tile([128, D], bf16, tag="ysb")
                nc.vector.tensor_copy(ysb[:], yp)
                nc.sync.dma_start(out=y[128 * tb:128 * (tb + 1), :],
                                  in_=ysb[:])

            def attention(pair, qc, inject=()):
                # heads 2*pair + hh, hh in {0,1} at partition offsets 0/64.
                # Inner loop is software pipelined: S_{j+1} is issued to the
                # PE queue before PV_j, so the in-order PE never stalls on
                # the exp of the S block it just produced.
                inject = list(inject)
                q0 = 512 * qc
                jmax = 4 * qc + 4
                oT = psO.tile([HD + 1, 2, 512], f32, tag="oT", name="oT")
                pts, avs = {}, {}
                for j in range(jmax + 1):
                    if j < jmax:
                        a = avs[j] = max(0, 128 * j - q0)
                        sT = psA.tile([128, 2, 512], f32, tag="ps",
                                      name="sT")
                        for hh in range(2):
                            o = 64 * hh
                            nc.tensor.matmul(
                                sT[:, hh, a:],
                                lhsT=kT[o:o + 64, pair,
                                        128 * j:128 * (j + 1)],
                                rhs=qT[o:o + 64, pair, q0 + a:q0 + 512],
                                start=True, stop=True)
                    if j > 0:
                        ap = avs.pop(j - 1)
                        pT = pts.pop(j - 1)
                        for hh in range(2):
                            nc.tensor.matmul(
                                oT[:, hh, ap:],
                                lhsT=vA[:, j - 1, 2 * pair + hh, :],
                                rhs=pT[:, hh, ap:],
                                start=(j == 1), stop=(j == jmax),
                                skip_group_check=True)
                    if j < jmax:
                        pT = pts[j] = ppool.tile([128, 2, 512], bf16,
                                                 tag="pt", name="pT")
                        nc.scalar.activation(pT[:, :, a:], sT[:, :, a:],
                                             Exp, scale=SM_SCALE)
                        if 128 * j >= q0:  # diagonal window, both heads
                            nc.vector.tensor_tensor(
                                pT[:, :, a:a + 128], pT[:, :, a:a + 128],
                                tri[:, None, :].to_broadcast((128, 2, 128)),
                                mult)
                    if inject:
                        inject.pop(0)()
                for fn in inject:
                    fn()
                # softmax denominators: row 64 of each head's O^T
                # (copy to SBUF first: custom-DVE recip can't read PSUM)
                srow = rcpool.tile([1, 2, 512], f32, tag="srow")
                nc.vector.tensor_copy(srow[:], oT[HD:HD + 1, :, :])
                nc.vector.reciprocal_approx_fast(srow[:], srow[:])
                srow_d = dpool.tile([1, 2, 512], f32, tag="srow_d")
                nc.gpsimd.dma_start(out=srow_d[:], in_=srow[:])
                rc = rcpool.tile([64, 2, 512], f32, tag="rc")
                sd = srow_d[:]
                bcast = bass.AP(sd.tensor, sd.offset,
                                [[0, 64]] + list(sd.ap[1:]))
                nc.gpsimd.dma_start(out=rc[:], in_=bcast)
                for hh in range(2):
                    nc.vector.tensor_tensor(
                        oTn[64 * hh:64 * hh + 64, pair, q0:q0 + 512],
                        oT[0:HD, hh, :], rc[:, hh, :], mult)

            # inject thunks allocate "ps" tiles in PAIRS (parity, see above)
            def pqk(pc, c):
                def f():
                    proj_qk(qT, wq_sb, xq_sb, bq_sb, pc, c)
                    proj_qk(kT, wk_sb, xk_sb, bk_sb, pc, c)
                return f

            def pv2(tb):
                def f():
                    proj_v(tb)
                    proj_v(tb + 1)
                return f

            def op2(tb):
                def f():
                    out_proj(tb)
                    out_proj(tb + 1)
                return f

            # ---- schedule ------------------------------------------------
            pqk(0, 0)()
            for tb in range(4):
                proj_v(tb)
            attention(0, 0, [pqk(0, 1), pv2(4), pv2(6)])
            attention(0, 1, [pqk(0, 2), pv2(8), pv2(10)])
            attention(0, 2, [pqk(0, 3), pv2(12), pv2(14)])
            attention(0, 3, [pqk(1, 0)])
            attention(1, 0, [pqk(1, 1)])
            attention(1, 1, [pqk(1, 2), op2(0), op2(2)])
            attention(1, 2, [pqk(1, 3), op2(4), op2(6)])
            attention(1, 3, [op2(8), op2(10)])
            for tb in range(12, 16):
                out_proj(tb)

    nc.compile()
    return nc


def _get_compiled():
    global _compiled
    if _compiled is None:
        _compiled = _build()
    return _compiled


def _reference_fallback(queries, keys, values, mask, Wq, bq, Wk, bk, Wv, bv,
                        Wo, bo):
    def split_heads(x):
        b, t, c = x.shape
        return x.reshape(b, t, H, c // H).transpose(0, 2, 1, 3)

    q = split_heads(queries @ Wq + bq)
    k = split_heads(keys @ Wk + bk)
    v = split_heads(values @ Wv + bv)
    wei = np.einsum("bhqd,bhkd->bhqk", q, k) * SM_SCALE
    wei = wei + (-1e9) * mask
    wei = wei - wei.max(axis=-1, keepdims=True)
    wei = np.exp(wei)
    wei = wei / wei.sum(axis=-1, keepdims=True)
    out = np.einsum("bhqk,bhkd->bhqd", wei, v)
    out = out.transpose(0, 2, 1, 3).reshape(queries.shape[0],
                                            queries.shape[1], D)
    return (out @ Wo + bo).astype(np.float32)


def kernel(queries, keys, values, mask, Wq, bq, Wk, bk, Wv, bv, Wo, bo):
    queries = np.asarray(queries, np.float32)
    keys = np.asarray(keys, np.float32)
    values = np.asarray(values, np.float32)
    Wq, Wk, Wv, Wo = (np.asarray(w, np.float32) for w in (Wq, Wk, Wv, Wo))
    bq, bk, bv, bo = (np.asarray(v_, np.float32) for v_ in (bq, bk, bv, bo))
    mask2d = np.asarray(mask, np.float32).reshape(T, T)
    causal = np.triu(np.ones((T, T), np.float32), k=1)
    if not np.array_equal(mask2d, causal):
        return _reference_fallback(queries, keys, values,
                                   np.asarray(mask, np.float32),
                                   Wq, bq, Wk, bk, Wv, bv, Wo, bo)

    from concourse.bass_utils import run_bass_kernel_spmd

    nc = _get_compiled()

    def bf(x):
        return np.ascontiguousarray(x).astype(_BF16)

    in_maps = []
    for core in range(N_CORES):
        b, g = core // 2, core % 2
        sl = slice(g * DG, (g + 1) * DG)
        in_maps.append({
            "xq_t": bf(queries[b].T),
            "xk_t": bf(keys[b].T),
            "xv_t": bf(values[b].T),
            "wq": bf(Wq[:, sl]),
            "wk": bf(Wk[:, sl]),
            "wv": bf(Wv[:, sl]),
            "wo": bf(Wo[sl, :]),
            "bq": np.ascontiguousarray(bq[sl].reshape(1, DG)),
            "bk": np.ascontiguousarray(bk[sl].reshape(1, DG)),
        })

    res = run_bass_kernel_spmd(nc, in_maps, list(range(N_CORES)))
    out = np.zeros((B, T, D), np.float32)
    for core in range(N_CORES):
        out[core // 2] += res.results[core]["y"].astype(np.float32)
    out += bo + bv @ Wo   # value bias is separable (softmax rows sum to 1)
    return out


# revision 15
# speedup vs baseline: 1.0257x; 1.0257x over previous
"""MultiHeadAttention Trainium2 kernel (8 NeuronCores).

Reference computation (B=4, T=2048, D=512, H=8, head_dim=64):
    q = split_heads(queries @ Wq + bq); k, v likewise
    wei = softmax(q k^T / sqrt(512) + (-1e9) * mask)   # mask: causal
    out = merge_heads(wei @ v) @ Wo + bo

Sharding: core = 2*b + g  (b in 0..3 batches, g in 0..1 groups of 4 heads).
Each core computes attention for its batch with its 4 heads plus a partial
output projection through its half of Wo's rows. Host sums the two partials
per batch and adds bo + bv @ Wo (the value bias is separable: softmax rows
sum to 1, so V += bv shifts each head's output by exactly bv).

fp8 path (Q/K only): the host packs X^T and Wq/Wk in fp8e4m3 with features
paired for the PE's DoubleRow mode (2 fp8 rows per cycle):
    x8 [2 s][128 p][2 i][T]      feature d = 256 s + 128 i + p
    w8 [2 s][128 p][2 i][2 pc][2 h][64 m]   out feature = fmap(h, m)
    fmap(h, P) = 64*(P//32) + 32*h + P%32   (head hh = P//32)
The Q/K projection runs as DoubleRow matmuls with M=64 halves landing in
PSUM [64, 2, 512], which the DVE bias-adds and casts straight into the
[64 P][2 pc][2 h][T] fp8 layout that the DoubleRow S-matmul consumes
(lhsT = k8[32hh:+32, pair], rhs = q8[32hh:+32, pair], PE row tiles 0/32).
Softmax logits therefore carry ~1% fp8 noise; V/PV/Wo stay bf16.

ScalarE runs ONLY the softmax exps (the per-core floor). PSUM->SBUF
copies (proj casts, V tiles, output tiles, O^T normalize, denominator row)
are on VectorE; gpsimd handles the k-side input DMA ring, masks and the
denominator DRAM-bounce broadcast. The attention inner loop is software
pipelined (S_{j+1} issued before PV_j) so the in-order PE queue never
waits on the exp it just produced; every emission keeps an EVEN number of
"ps"-tag PSUM allocations so the S tiles stay double-buffered.
"""

import sys

if "/opt/trn_rl_repo" not in sys.path:
    sys.path.insert(0, "/opt/trn_rl_repo")

import numpy as np
import ml_dtypes

B, T, D, H = 4, 2048, 512, 8
HPG = 4                 # heads per group (per core)
HD = 64                 # head dim
DG = HPG * HD           # 256 feature dims per group
N_CORES = 8
SM_SCALE = float(D) ** -0.5   # module scales by full d_k = 512

_BF16 = ml_dtypes.bfloat16
_FP8 = ml_dtypes.float8_e4m3

# out partition P of proj half h -> feature index within the 128-col block
_FMAP = np.array([[64 * (P // 32) + 32 * h + P % 32 for P in range(64)]
                  for h in range(2)])          # [h, P]
# [pc, h, m] -> column within the 256-col group
_COLMAP = np.array([[[128 * pc + _FMAP[h, m] for m in range(64)]
                     for h in range(2)] for pc in range(2)])

_compiled = None


def _build():
    import concourse.bass as bass
    import concourse.bacc as bacc
    import concourse.tile as tile
    import concourse.mybir as mybir

    f32 = mybir.dt.float32
    bf16 = mybir.dt.bfloat16
    fp8 = mybir.dt.float8e4
    Exp = mybir.ActivationFunctionType.Exp
    add = mybir.AluOpType.add
    mult = mybir.AluOpType.mult
    DR = mybir.MatmulPerfMode.DoubleRow

    nc = bacc.Bacc("TRN2", target_bir_lowering=False, debug=False,
                   num_devices=N_CORES)

    xq8 = nc.dram_tensor("xq8", [2, 128, 2, T], fp8,
                         kind="ExternalInput").ap()
    xk8 = nc.dram_tensor("xk8", [2, 128, 2, T], fp8,
                         kind="ExternalInput").ap()
    xv = nc.dram_tensor("xv_t", [D, T], bf16, kind="ExternalInput").ap()
    wq8 = nc.dram_tensor("wq8", [2, 128, 2, 2, 2, 64], fp8,
                         kind="ExternalInput").ap()
    wk8 = nc.dram_tensor("wk8", [2, 128, 2, 2, 2, 64], fp8,
                         kind="ExternalInput").ap()
    wv = nc.dram_tensor("wv", [D, DG], bf16, kind="ExternalInput").ap()
    wo = nc.dram_tensor("wo", [DG, D], bf16, kind="ExternalInput").ap()
    bq8 = nc.dram_tensor("bq8", [64, 2, 2], f32, kind="ExternalInput").ap()
    bk8 = nc.dram_tensor("bk8", [64, 2, 2], f32, kind="ExternalInput").ap()
    y = nc.dram_tensor("y", [T, D], bf16, kind="ExternalOutput").ap()

    NT = T // 512        # 4 t/q chunks of 512
    NB = T // 128        # 16 t/k blocks of 128
    ND = D // 128        # 4 contraction chunks over D

    with tile.TileContext(nc) as tc:
        with (
            tc.tile_pool(name="const", bufs=1) as const,
            tc.tile_pool(name="pt", bufs=10) as ppool,
            tc.tile_pool(name="rc", bufs=4) as rcpool,
            tc.tile_pool(name="ysb", bufs=6) as ypool,
            tc.tile_pool(name="psA", bufs=2, space="PSUM") as psA,
            tc.tile_pool(name="psO", bufs=2, space="PSUM") as psO,
            tc.tile_pool(name="dscratch", bufs=2, space="DRAM") as dpool,
        ):
            # ---- input DMAs: sync ring for q/v side, gpsimd for k side --
            bq_sb = const.tile([64, 2, 2], f32, tag="bq")
            nc.sync.dma_start(out=bq_sb[:], in_=bq8)
            bk_sb = const.tile([64, 2, 2], f32, tag="bk")
            nc.sync.dma_start(out=bk_sb[:], in_=bk8)
            wq_sb = const.tile([128, 2, 2, 2, 2, 64], fp8, tag="wq")
            nc.sync.dma_start(
                out=wq_sb[:], in_=wq8.rearrange("s p i c h m -> p s i c h m"))
            wk_sb = const.tile([128, 2, 2, 2, 2, 64], fp8, tag="wk")
            nc.gpsimd.dma_start(
                out=wk_sb[:], in_=wk8.rearrange("s p i c h m -> p s i c h m"))

            xq_sb = const.tile([128, 2, 2, T], fp8, tag="xq")
            xk_sb = const.tile([128, 2, 2, T], fp8, tag="xk")
            xv_sb = [const.tile([128, T], bf16, tag=f"xv{dc}",
                                name=f"xv{dc}") for dc in range(ND)]
            wv_sb = const.tile([128, ND, DG], bf16, tag="wv")
            wo_sb = const.tile([128, 2, D], bf16, tag="wo")

            def load_x8(sb, dram, h, eng):
                s = slice(1024 * h, 1024 * (h + 1))
                for sc in range(2):
                    eng.dma_start(out=sb[:, sc, :, s],
                                  in_=dram[sc, :, :, s])

            load_x8(xq_sb, xq8, 0, nc.sync)
            load_x8(xk_sb, xk8, 0, nc.gpsimd)
            nc.sync.dma_start(
                out=wv_sb[:], in_=wv.rearrange("(c p) m -> p c m", p=128))
            for dc in range(ND):
                nc.sync.dma_start(out=xv_sb[dc][:, 0:1024],
                                  in_=xv[128 * dc:128 * (dc + 1), 0:1024])
            load_x8(xq_sb, xq8, 1, nc.sync)
            load_x8(xk_sb, xk8, 1, nc.gpsimd)
            for dc in range(ND):
                nc.sync.dma_start(out=xv_sb[dc][:, 1024:2048],
                                  in_=xv[128 * dc:128 * (dc + 1), 1024:2048])
            nc.sync.dma_start(out=wo_sb[:],
                              in_=wo.rearrange("(c p) n -> p c n", p=128))

            # PE warm-up while the DMAs land: open the HAM clock gate
            # (1.2 -> 2.4 GHz needs ~3us of continuous execution).
            # NOTE every emission keeps an EVEN number of "ps"-tag psum
            # allocations so the attention S tiles stay double-buffered.
            warm = const.tile([128, 512], bf16, tag="warm")
            nc.vector.memset(warm[:], 0.0)
            for _ in range(2):
                wps = psA.tile([128, 1024], f32, tag="ps", name="wps")
                for _ in range(14):
                    nc.tensor.matmul(wps[:, :512], lhsT=warm[:, :128],
                                     rhs=warm[:], start=True, stop=True)

            # 0/1 lower-triangular (keep k <= q): tri[k, q] = (q - k >= 0)
            tri = const.tile([128, 128], bf16, tag="tri")
            nc.gpsimd.memset(tri[:], 1.0)
            nc.gpsimd.affine_select(
                out=tri[:], in_=tri[:],
                compare_op=mybir.AluOpType.is_ge, fill=0.0,
                base=0, pattern=[[1, 128]], channel_multiplier=-1)

            q8 = const.tile([64, 2, 2, T], fp8, tag="q8")
            k8 = const.tile([64, 2, 2, T], fp8, tag="k8")
            vA = const.tile([128, NB, HPG, HD + 1], bf16, tag="vA")
            nc.gpsimd.memset(vA[:, :, :, HD:HD + 1], 1.0)
            oTn = const.tile([128, 2, T], bf16, tag="oTn")

            def proj_qk(dst8, w_sb, x_sb, b_sb, pc, c):
                # one 512-wide t-chunk of Q^T / K^T for head pair pc,
                # DoubleRow fp8, M=64 halves -> PSUM [64, 2, 512]
                ps = psA.tile([64, 2, 512], f32, tag="ps", name="ps")
                for h in range(2):
                    for s in range(2):
                        nc.tensor.matmul(
                            ps[:, h, :],
                            lhsT=w_sb[:, s, :, pc, h, :],
                            rhs=x_sb[:, s, :, 512 * c:512 * (c + 1)],
                            start=(s == 0), stop=(s == 1),
                            perf_mode=DR)
                # bias-add + fp8 cast on VectorE (keeps ScalarE exp-only)
                nc.vector.scalar_tensor_tensor(
                    dst8[:, pc, :, 512 * c:512 * (c + 1)], ps[:], 1.0,
                    b_sb[:, pc, :, None].to_broadcast((64, 2, 512)),
                    mult, add)

            def proj_v(tb):
                ps = psA.tile([128, 1024], f32, tag="ps",
                              name="vps")[:, :DG]
                for dc in range(ND):
                    nc.tensor.matmul(
                        ps,
                        lhsT=xv_sb[dc][:, 128 * tb:128 * (tb + 1)],
                        rhs=wv_sb[:, dc, :],
                        start=(dc == 0), stop=(dc == ND - 1))
                nc.vector.tensor_copy(
                    vA[:, tb, :, 0:HD],
                    ps.rearrange("p (h d) -> p h d", h=HPG))

            def out_proj(tb):
                yp = psA.tile([128, 1024], f32, tag="ps",
                              name="yp")[:, :512]
                for pair in range(2):
                    nc.tensor.matmul(
                        yp,
                        lhsT=oTn[:, pair, 128 * tb:128 * (tb + 1)],
                        rhs=wo_sb[:, pair, :],
                        start=(pair == 0), stop=(pair == 1))
                ysb = ypool.tile([128, D], bf16, tag="ysb")
                nc.vector.tensor_copy(ysb[:], yp)
                nc.sync.dma_start(out=y[128 * tb:128 * (tb + 1), :],
                                  in_=ysb[:])

            def attention(pair, qc, inject=()):
                # heads 2*pair + hh, hh in {0,1}; PE row tiles 32hh (S).
                # Software pipelined: S_{j+1} is issued to the PE queue
                # before PV_j, so the in-order PE never stalls on the exp
                # of the S block it just produced.
                inject = list(inject)
                q0 = 512 * qc
                jmax = 4 * qc + 4
                oT = psO.tile([HD + 1, 2, 512], f32, tag="oT", name="oT")
                pts, avs = {}, {}
                for j in range(jmax + 1):
                    if j < jmax:
                        a = avs[j] = max(0, 128 * j - q0)
                        sT = psA.tile([128, 2, 512], f32, tag="ps",
                                      name="sT")
                        for hh in range(2):
                            o = 32 * hh
                            nc.tensor.matmul(
                                sT[:, hh, a:],
                                lhsT=k8[o:o + 32, pair, :,
                                        128 * j:128 * (j + 1)],
                                rhs=q8[o:o + 32, pair, :, q0 + a:q0 + 512],
                                start=True, stop=True,
                                perf_mode=DR)
                    if j > 0:
                        ap = avs.pop(j - 1)
                        pT = pts.pop(j - 1)
                        for hh in range(2):
                            nc.tensor.matmul(
                                oT[:, hh, ap:],
                                lhsT=vA[:, j - 1, 2 * pair + hh, :],
                                rhs=pT[:, hh, ap:],
                                start=(j == 1), stop=(j == jmax),
                                skip_group_check=True)
                    if j < jmax:
                        pT = pts[j] = ppool.tile([128, 2, 512], bf16,
                                                 tag="pt", name="pT")
                        nc.scalar.activation(pT[:, :, a:], sT[:, :, a:],
                                             Exp, scale=SM_SCALE)
                        if 128 * j >= q0:  # diagonal window, both heads
                            nc.vector.tensor_tensor(
                                pT[:, :, a:a + 128], pT[:, :, a:a + 128],
                                tri[:, None, :].to_broadcast((128, 2, 128)),
                                mult)
                    if inject:
                        inject.pop(0)()
                for fn in inject:
                    fn()
                # softmax denominators: row 64 of each head's O^T
                # (copy to SBUF first: custom-DVE recip can't read PSUM)
                srow = rcpool.tile([1, 2, 512], f32, tag="srow")
                nc.vector.tensor_copy(srow[:], oT[HD:HD + 1, :, :])
                nc.vector.reciprocal_approx_fast(srow[:], srow[:])
                srow_d = dpool.tile([1, 2, 512], f32, tag="srow_d")
                nc.gpsimd.dma_start(out=srow_d[:], in_=srow[:])
                rc = rcpool.tile([64, 2, 512], f32, tag="rc")
                sd = srow_d[:]
                bcast = bass.AP(sd.tensor, sd.offset,
                                [[0, 64]] + list(sd.ap[1:]))
                nc.gpsimd.dma_start(out=rc[:], in_=bcast)
                for hh in range(2):
                    nc.vector.tensor_tensor(
                        oTn[64 * hh:64 * hh + 64, pair, q0:q0 + 512],
                        oT[0:HD, hh, :], rc[:, hh, :], mult)

            # inject thunks allocate "ps" tiles in PAIRS (parity, see above)
            def pqk(pc, c):
                def f():
                    proj_qk(q8, wq_sb, xq_sb, bq_sb, pc, c)
                    proj_qk(k8, wk_sb, xk_sb, bk_sb, pc, c)
                return f

            def pv2(tb):
                def f():
                    proj_v(tb)
                    proj_v(tb + 1)
                return f

            def op2(tb):
                def f():
                    out_proj(tb)
                    out_proj(tb + 1)
                return f

            # ---- schedule ------------------------------------------------
            pqk(0, 0)()
            attention(0, 0, [pv2(0), pv2(2), pqk(0, 1), pv2(4), pv2(6)])
            attention(0, 1, [pqk(0, 2), pv2(8), pv2(10)])
            attention(0, 2, [pqk(0, 3), pv2(12), pv2(14)])
            attention(0, 3, [pqk(1, 0)])
            attention(1, 0, [pqk(1, 1)])
            attention(1, 1, [pqk(1, 2), op2(0), op2(2)])
            attention(1, 2, [pqk(1, 3), op2(4), op2(6)])
            attention(1, 3, [op2(8), op2(10)])
            for tb in range(12, 16):
                out_proj(tb)

    nc.compile()
    return nc


def _get_compiled():
    global _compiled
    if _compiled is None:
        _compiled = _build()
    return _compiled


def _pack_x8(x):
    """X [T, 512] -> [2 s, 128 p, 2 i, T] fp8 (feature d = 256s+128i+p)."""
    xt = np.ascontiguousarray(x.T).astype(_FP8)        # [512, T]
    return np.ascontiguousarray(
        xt.reshape(2, 2, 128, T).transpose(0, 2, 1, 3))


def _pack_w8(w, g):
    """W [512, 512] -> [2 s, 128 p, 2 i, 2 pc, 2 h, 64 m] fp8 for group g."""
    wg = w[:, DG * g:DG * (g + 1)].astype(_FP8)        # [512, 256]
    wr = wg.reshape(2, 2, 128, DG)                     # [s, i, p, col]
    out = wr[:, :, :, _COLMAP]                         # [s, i, p, pc, h, m]
    return np.ascontiguousarray(out.transpose(0, 2, 1, 3, 4, 5))


def _pack_b8(bvec, g):
    """b [512] -> [64 P, 2 pc, 2 h] f32 for group g."""
    out = np.asarray(bvec, np.float32)[DG * g:][_COLMAP]   # [pc, h, P]
    return np.ascontiguousarray(out.transpose(2, 0, 1))


def make_in_maps(queries, keys, values, Wq, bq, Wk, bk, Wv, Wo):
    def bf(x):
        return np.ascontiguousarray(x).astype(_BF16)

    xq8 = [_pack_x8(queries[b]) for b in range(B)]
    xk8 = [_pack_x8(keys[b]) for b in range(B)]
    xvt = [bf(values[b].T) for b in range(B)]
    in_maps = []
    for core in range(N_CORES):
        b, g = core // 2, core % 2
        sl = slice(g * DG, (g + 1) * DG)
        in_maps.append({
            "xq8": xq8[b],
            "xk8": xk8[b],
            "xv_t": xvt[b],
            "wq8": _pack_w8(Wq, g),
            "wk8": _pack_w8(Wk, g),
            "wv": bf(Wv[:, sl]),
            "wo": bf(Wo[sl, :]),
            "bq8": _pack_b8(bq, g),
            "bk8": _pack_b8(bk, g),
        })
    return in_maps


def _reference_fallback(queries, keys, values, mask, Wq, bq, Wk, bk, Wv, bv,
                        Wo, bo):
    def split_heads(x):
        b, t, c = x.shape
        return x.reshape(b, t, H, c // H).transpose(0, 2, 1, 3)

    q = split_heads(queries @ Wq + bq)
    k = split_heads(keys @ Wk + bk)
    v = split_heads(values @ Wv + bv)
    wei = np.einsum("bhqd,bhkd->bhqk", q, k) * SM_SCALE
    wei = wei + (-1e9) * mask
    wei = wei - wei.max(axis=-1, keepdims=True)
    wei = np.exp(wei)
    wei = wei / wei.sum(axis=-1, keepdims=True)
    out = np.einsum("bhqk,bhkd->bhqd", wei, v)
    out = out.transpose(0, 2, 1, 3).reshape(queries.shape[0],
                                            queries.shape[1], D)
    return (out @ Wo + bo).astype(np.float32)


def kernel(queries, keys, values, mask, Wq, bq, Wk, bk, Wv, bv, Wo, bo):
    queries = np.asarray(queries, np.float32)
    keys = np.asarray(keys, np.float32)
    values = np.asarray(values, np.float32)
    Wq, Wk, Wv, Wo = (np.asarray(w, np.float32) for w in (Wq, Wk, Wv, Wo))
    bq, bk, bv, bo = (np.asarray(v_, np.float32) for v_ in (bq, bk, bv, bo))
    mask2d = np.asarray(mask, np.float32).reshape(T, T)
    causal = np.triu(np.ones((T, T), np.float32), k=1)
    if not np.array_equal(mask2d, causal):
        return _reference_fallback(queries, keys, values,
                                   np.asarray(mask, np.float32),
                                   Wq, bq, Wk, bk, Wv, bv, Wo, bo)

    from concourse.bass_utils import run_bass_kernel_spmd

    nc = _get_compiled()
    in_maps = make_in_maps(queries, keys, values, Wq, bq, Wk, bk, Wv, Wo)
    res = run_bass_kernel_spmd(nc, in_maps, list(range(N_CORES)))
    out = np.zeros((B, T, D), np.float32)
    for core in range(N_CORES):
        out[core // 2] += res.results[core]["y"].astype(np.float32)
    out += bo + bv @ Wo   # value bias is separable (softmax rows sum to 1)
    return out


# revision 32
# speedup vs baseline: 1.2870x; 1.2547x over previous
"""MultiHeadAttention Trainium2 kernel (8 NeuronCores).

Reference computation (B=4, T=2048, D=512, H=8, head_dim=64):
    q = split_heads(queries @ Wq + bq); k, v likewise
    wei = softmax(q k^T / sqrt(512) + (-1e9) * mask)   # mask: causal
    out = merge_heads(wei @ v) @ Wo + bo

Sharding: core = 2*b + g  (b in 0..3 batches, g in 0..1 groups of 4 heads).
Each core computes attention for its batch with its 4 heads plus a partial
output projection through its half of Wo's rows. Host sums the two partials
per batch and adds bo + bv @ Wo (the value bias is separable: softmax rows
sum to 1, so V += bv shifts each head's output by exactly bv).

fp8 path (Q/K only): the host packs X^T and Wq/Wk in fp8e4m3 with features
paired for the PE's DoubleRow mode (2 fp8 rows per cycle):
    x8 [2 s][128 p][2 i][T]      feature d = 256 s + 128 i + p
    w8 [2 s][128 p][2 i][2 pc][2 h][64 m]   out feature = fmap(h, m)
    fmap(h, P) = 64*(P//32) + 32*h + P%32   (head hh = P//32)
The Q/K projection runs as DoubleRow matmuls with M=64 halves landing in
PSUM [64, 2, 512], which the DVE bias-adds and casts straight into the
[64 P][2 pc][2 h][T] fp8 layout that the DoubleRow S-matmul consumes
(lhsT = k8[32hh:+32, pair], rhs = q8[32hh:+32, pair], PE row tiles 0/32).
Softmax logits therefore carry ~1% fp8 noise; V/PV/Wo stay bf16.

ScalarE runs ONLY the softmax exps (the per-core floor). PSUM->SBUF
copies (proj casts, V tiles, output tiles, O^T normalize, denominator row)
are on VectorE; gpsimd handles the k-side input DMA ring, masks and the
denominator DRAM-bounce broadcast. The attention inner loop is software
pipelined (S_{j+1} issued before PV_j) so the in-order PE queue never
waits on the exp it just produced; every emission keeps an EVEN number of
"ps"-tag PSUM allocations so the S tiles stay double-buffered.
"""

import sys

if "/opt/trn_rl_repo" not in sys.path:
    sys.path.insert(0, "/opt/trn_rl_repo")

import numpy as np
import ml_dtypes

B, T, D, H = 4, 2048, 512, 8
HPG = 4                 # heads per group (per core)
HD = 64                 # head dim
DG = HPG * HD           # 256 feature dims per group
N_CORES = 8
SM_SCALE = float(D) ** -0.5   # module scales by full d_k = 512

_BF16 = ml_dtypes.bfloat16
_FP8 = ml_dtypes.float8_e4m3

# out partition P of proj half h -> feature index within the 128-col block
_FMAP = np.array([[64 * (P // 32) + 32 * h + P % 32 for P in range(64)]
                  for h in range(2)])          # [h, P]
# [pc, h, m] -> column within the 256-col group
_COLMAP = np.array([[[128 * pc + _FMAP[h, m] for m in range(64)]
                     for h in range(2)] for pc in range(2)])

_compiled = None


def _build():
    import concourse.bass as bass
    import concourse.bacc as bacc
    import concourse.tile as tile
    import concourse.mybir as mybir

    f32 = mybir.dt.float32
    bf16 = mybir.dt.bfloat16
    fp8 = mybir.dt.float8e4
    Exp = mybir.ActivationFunctionType.Exp
    add = mybir.AluOpType.add
    mult = mybir.AluOpType.mult
    DR = mybir.MatmulPerfMode.DoubleRow

    nc = bacc.Bacc("TRN2", target_bir_lowering=False, debug=False,
                   num_devices=N_CORES)

    xq8 = nc.dram_tensor("xq8", [2, 128, 2, T], fp8,
                         kind="ExternalInput").ap()
    xk8 = nc.dram_tensor("xk8", [2, 128, 2, T], fp8,
                         kind="ExternalInput").ap()
    xv = nc.dram_tensor("xv_t", [D, T], bf16, kind="ExternalInput").ap()
    wq8 = nc.dram_tensor("wq8", [2, 128, 2, 2, 2, 64], fp8,
                         kind="ExternalInput").ap()
    wk8 = nc.dram_tensor("wk8", [2, 128, 2, 2, 2, 64], fp8,
                         kind="ExternalInput").ap()
    wv = nc.dram_tensor("wv", [D, DG], bf16, kind="ExternalInput").ap()
    wo = nc.dram_tensor("wo", [DG, D], bf16, kind="ExternalInput").ap()
    bq8 = nc.dram_tensor("bq8", [64, 2, 2], f32, kind="ExternalInput").ap()
    bk8 = nc.dram_tensor("bk8", [64, 2, 2], f32, kind="ExternalInput").ap()
    y = nc.dram_tensor("y", [T, D], bf16, kind="ExternalOutput").ap()

    NT = T // 512        # 4 t/q chunks of 512
    NB = T // 128        # 16 t/k blocks of 128
    ND = D // 128        # 4 contraction chunks over D

    with tile.TileContext(nc) as tc:
        with (
            tc.tile_pool(name="const", bufs=1) as const,
            tc.tile_pool(name="pt", bufs=10) as ppool,
            tc.tile_pool(name="rc", bufs=4) as rcpool,
            tc.tile_pool(name="ysb", bufs=6) as ypool,
            tc.tile_pool(name="psA", bufs=2, space="PSUM") as psA,
            tc.tile_pool(name="psO", bufs=2, space="PSUM") as psO,
            tc.tile_pool(name="dscratch", bufs=2, space="DRAM") as dpool,
        ):
            # ---- input DMAs: sync ring for q/v side, gpsimd for k side --
            bq_sb = const.tile([64, 2, 2], f32, tag="bq")
            nc.sync.dma_start(out=bq_sb[:], in_=bq8)
            bk_sb = const.tile([64, 2, 2], f32, tag="bk")
            nc.sync.dma_start(out=bk_sb[:], in_=bk8)
            wq_sb = const.tile([128, 2, 2, 2, 2, 64], fp8, tag="wq")
            nc.sync.dma_start(
                out=wq_sb[:], in_=wq8.rearrange("s p i c h m -> p s i c h m"))
            wk_sb = const.tile([128, 2, 2, 2, 2, 64], fp8, tag="wk")
            nc.gpsimd.dma_start(
                out=wk_sb[:], in_=wk8.rearrange("s p i c h m -> p s i c h m"))

            xq_sb = const.tile([128, 2, 2, T], fp8, tag="xq")
            xk_sb = const.tile([128, 2, 2, T], fp8, tag="xk")
            xv_sb = [const.tile([128, T], bf16, tag=f"xv{dc}",
                                name=f"xv{dc}") for dc in range(ND)]
            wv_sb = const.tile([128, ND, DG], bf16, tag="wv")
            wo_sb = const.tile([128, 2, D], bf16, tag="wo")

            def load_x8(sb, dram, lo, hi, eng):
                s = slice(lo, hi)
                for sc in range(2):
                    eng.dma_start(out=sb[:, sc, :, s],
                                  in_=dram[sc, :, :, s])

            # chunk 0 of q/k first: it alone gates the first S block
            load_x8(xq_sb, xq8, 0, 512, nc.sync)
            load_x8(xk_sb, xk8, 0, 512, nc.gpsimd)
            nc.sync.dma_start(
                out=wv_sb[:], in_=wv.rearrange("(c p) m -> p c m", p=128))
            for dc in range(ND):
                nc.sync.dma_start(out=xv_sb[dc][:, 0:1024],
                                  in_=xv[128 * dc:128 * (dc + 1), 0:1024])
            load_x8(xq_sb, xq8, 512, 1024, nc.sync)
            load_x8(xk_sb, xk8, 512, 1024, nc.gpsimd)
            load_x8(xq_sb, xq8, 1024, 2048, nc.sync)
            load_x8(xk_sb, xk8, 1024, 2048, nc.gpsimd)
            for dc in range(ND):
                nc.sync.dma_start(out=xv_sb[dc][:, 1024:2048],
                                  in_=xv[128 * dc:128 * (dc + 1), 1024:2048])
            nc.sync.dma_start(out=wo_sb[:],
                              in_=wo.rearrange("(c p) n -> p c n", p=128))

            # tiny PE warm-up while the DMAs land (a big block would delay
            # the first projection: the PE queue is in-order).
            # NOTE every emission keeps an EVEN number of "ps"-tag psum
            # allocations so the attention S tiles stay double-buffered.
            warm = const.tile([128, 512], bf16, tag="warm")
            nc.vector.memset(warm[:], 0.0)
            for _ in range(2):
                wps = psA.tile([128, 1024], f32, tag="ps", name="wps")
                for _ in range(4):
                    nc.tensor.matmul(wps[:, :512], lhsT=warm[:, :128],
                                     rhs=warm[:], start=True, stop=True)

            # causal masking runs on the PE: the diagonal S window gets
            # ident^T @ negtri accumulated into it (adds -1e5 where q < k,
            # so the exp underflows to exactly 0 and PV needs no mask).
            ident = const.tile([128, 128], bf16, tag="ident")
            nc.gpsimd.memset(ident[:], 1.0)
            nc.gpsimd.affine_select(
                out=ident[:], in_=ident[:],
                compare_op=mybir.AluOpType.is_ge, fill=0.0,
                base=0, pattern=[[1, 128]], channel_multiplier=-1)
            nc.gpsimd.affine_select(
                out=ident[:], in_=ident[:],
                compare_op=mybir.AluOpType.is_ge, fill=0.0,
                base=0, pattern=[[-1, 128]], channel_multiplier=1)
            negtri = const.tile([128, 128], bf16, tag="negtri")
            nc.gpsimd.memset(negtri[:], 0.0)
            nc.gpsimd.affine_select(
                out=negtri[:], in_=negtri[:],
                compare_op=mybir.AluOpType.is_ge, fill=-1e5,
                base=0, pattern=[[1, 128]], channel_multiplier=-1)
            # 0/1 lower-triangular for the DVE-side mask (even diagonals)
            tri = const.tile([128, 128], bf16, tag="tri")
            nc.gpsimd.memset(tri[:], 1.0)
            nc.gpsimd.affine_select(
                out=tri[:], in_=tri[:],
                compare_op=mybir.AluOpType.is_ge, fill=0.0,
                base=0, pattern=[[1, 128]], channel_multiplier=-1)

            q8 = const.tile([64, 2, 2, T], fp8, tag="q8")
            k8 = const.tile([64, 2, 2, T], fp8, tag="k8")
            vA = const.tile([128, NB, HPG, HD + 1], bf16, tag="vA")
            nc.gpsimd.memset(vA[:, :, :, HD:HD + 1], 1.0)
            oTn = const.tile([128, 2, T], bf16, tag="oTn")

            def proj_qk(dst8, w_sb, x_sb, b_sb, pc, c):
                # one 512-wide t-chunk of Q^T / K^T for head pair pc,
                # DoubleRow fp8, M=64 halves -> PSUM [64, 2, 512]
                ps = psA.tile([64, 2, 512], f32, tag="ps", name="ps")
                for h in range(2):
                    for s in range(2):
                        nc.tensor.matmul(
                            ps[:, h, :],
                            lhsT=w_sb[:, s, :, pc, h, :],
                            rhs=x_sb[:, s, :, 512 * c:512 * (c + 1)],
                            start=(s == 0), stop=(s == 1),
                            perf_mode=DR)
                # bias-add + fp8 cast on VectorE (keeps ScalarE exp-only)
                nc.vector.scalar_tensor_tensor(
                    dst8[:, pc, :, 512 * c:512 * (c + 1)], ps[:], 1.0,
                    b_sb[:, pc, :, None].to_broadcast((64, 2, 512)),
                    mult, add)

            def proj_v(tb):
                # two adjacent t-blocks per psum tile: one merged DVE copy
                ps = psA.tile([128, 2, DG], f32, tag="ps", name="vps")
                for s in range(2):
                    for dc in range(ND):
                        nc.tensor.matmul(
                            ps[:, s, :],
                            lhsT=xv_sb[dc][:, 128 * (tb + s):
                                           128 * (tb + s + 1)],
                            rhs=wv_sb[:, dc, :],
                            start=(dc == 0), stop=(dc == ND - 1))
                nc.vector.tensor_copy(
                    vA[:, tb:tb + 2, :, 0:HD],
                    ps.rearrange("p s (h d) -> p s h d", h=HPG))

            def out_proj(tb):
                # two adjacent t-blocks per psum tile: one copy, one DMA
                yp = psA.tile([128, 2, D], f32, tag="ps", name="yp")
                for s in range(2):
                    for pair in range(2):
                        nc.tensor.matmul(
                            yp[:, s, :],
                            lhsT=oTn[:, pair, 128 * (tb + s):
                                     128 * (tb + s + 1)],
                            rhs=wo_sb[:, pair, :],
                            start=(pair == 0), stop=(pair == 1))
                ysb = ypool.tile([128, 2, D], bf16, tag="ysb")
                nc.vector.tensor_copy(ysb[:], yp[:])
                nc.sync.dma_start(
                    out=y[128 * tb:128 * (tb + 2), :].rearrange(
                        "(s p) d -> p s d", p=128),
                    in_=ysb[:])

            def attention(pair, qc, inject=(), lag=2):
                # heads 2*pair + hh, hh in {0,1}; PE row tiles 32hh (S).
                # Software pipelined: PV_j is issued `lag` S-blocks after
                # S_j, so the in-order PE queue never stalls the exp feed
                # on a PV whose inputs (vA, pT) are late. inject =
                # [(min_j, fn)]: fn is emitted once j >= min_j (work whose
                # inputs are only ready later never head-of-line blocks).
                inject = list(inject)
                q0 = 512 * qc
                jmax = 4 * qc + 4
                oT = psO.tile([HD + 1, 2, 512], f32, tag="oT", name="oT")
                pts, avs = {}, {}
                for j in range(jmax + lag):
                    if j < jmax:
                        a = avs[j] = max(0, 128 * j - q0)
                        diag = 128 * j >= q0
                        pe_mask = diag and (j % 2 == 1)
                        sT = psA.tile([128, 2, 512], f32, tag="ps",
                                      name="sT")
                        for hh in range(2):
                            o = 32 * hh
                            nc.tensor.matmul(
                                sT[:, hh, a:],
                                lhsT=k8[o:o + 32, pair, :,
                                        128 * j:128 * (j + 1)],
                                rhs=q8[o:o + 32, pair, :, q0 + a:q0 + 512],
                                start=True, stop=not pe_mask,
                                perf_mode=DR,
                                skip_group_check=pe_mask)
                        if pe_mask:  # add -1e5 above the diagonal (PE)
                            for hh in range(2):
                                nc.tensor.matmul(
                                    sT[:, hh, a:a + 128],
                                    lhsT=ident[:],
                                    rhs=negtri[:],
                                    start=False, stop=True,
                                    skip_group_check=True)
                    if j >= lag:
                        jv = j - lag
                        ap = avs.pop(jv)
                        pT = pts.pop(jv)
                        for hh in range(2):
                            nc.tensor.matmul(
                                oT[:, hh, ap:],
                                lhsT=vA[:, jv, 2 * pair + hh, :],
                                rhs=pT[:, hh, ap:],
                                start=(jv == 0), stop=(jv == jmax - 1),
                                skip_group_check=True)
                    if j < jmax:
                        pT = pts[j] = ppool.tile([128, 2, 512], bf16,
                                                 tag="pt", name="pT")
                        nc.scalar.activation(pT[:, :, a:], sT[:, :, a:],
                                             Exp, scale=SM_SCALE)
                        if diag and not pe_mask:  # DVE-side mask
                            nc.vector.tensor_tensor(
                                pT[:, :, a:a + 128], pT[:, :, a:a + 128],
                                tri[:, None, :].to_broadcast((128, 2, 128)),
                                mult)
                    while inject and j >= inject[0][0]:
                        inject.pop(0)[1]()
                for _, fn in inject:
                    fn()
                # softmax denominators: row 64 of each head's O^T
                # (copy to SBUF first: custom-DVE recip can't read PSUM)
                srow = rcpool.tile([1, 2, 512], f32, tag="srow")
                nc.vector.tensor_copy(srow[:], oT[HD:HD + 1, :, :])
                nc.vector.reciprocal_approx_fast(srow[:], srow[:])
                rc = rcpool.tile([64, 2, 512], f32, tag="rc")
                nc.gpsimd.partition_broadcast(rc[:], srow[:], channels=64)
                for hh in range(2):
                    nc.vector.tensor_tensor(
                        oTn[64 * hh:64 * hh + 64, pair, q0:q0 + 512],
                        oT[0:HD, hh, :], rc[:, hh, :], mult)

            # inject thunks allocate "ps" tiles in PAIRS (parity, see above)
            def pqk(pc, c):
                def f():
                    proj_qk(q8, wq_sb, xq_sb, bq_sb, pc, c)
                    proj_qk(k8, wk_sb, xk_sb, bk_sb, pc, c)
                return f

            def pv4(tb):
                def f():
                    proj_v(tb)
                    proj_v(tb + 2)
                return f

            def op4(tb):
                def f():
                    out_proj(tb)
                    out_proj(tb + 2)
                return f

            # ---- schedule ------------------------------------------------
            pqk(0, 0)()
            attention(0, 0, [(2, pqk(0, 1)), (3, pv4(0))], lag=4)
            attention(0, 1, [(0, pv4(4)), (2, pqk(0, 2))])
            attention(0, 2, [(0, pv4(8)), (2, pqk(0, 3)), (6, pv4(12))])
            attention(0, 3, [(0, pqk(1, 0))])
            attention(1, 0, [(0, pqk(1, 1))])
            attention(1, 1, [(0, pqk(1, 2)), (5, op4(0))])
            attention(1, 2, [(0, pqk(1, 3)), (5, op4(4))])
            attention(1, 3, [(5, op4(8))])
            op4(12)()

    nc.compile()
    return nc


def _get_compiled():
    global _compiled
    if _compiled is None:
        _compiled = _build()
    return _compiled


def _pack_x8(x):
    """X [T, 512] -> [2 s, 128 p, 2 i, T] fp8 (feature d = 256s+128i+p)."""
    xt = np.ascontiguousarray(x.T).astype(_FP8)        # [512, T]
    return np.ascontiguousarray(
        xt.reshape(2, 2, 128, T).transpose(0, 2, 1, 3))


def _pack_w8(w, g):
    """W [512, 512] -> [2 s, 128 p, 2 i, 2 pc, 2 h, 64 m] fp8 for group g."""
    wg = w[:, DG * g:DG * (g + 1)].astype(_FP8)        # [512, 256]
    wr = wg.reshape(2, 2, 128, DG)                     # [s, i, p, col]
    out = wr[:, :, :, _COLMAP]                         # [s, i, p, pc, h, m]
    return np.ascontiguousarray(out.transpose(0, 2, 1, 3, 4, 5))


def _pack_b8(bvec, g):
    """b [512] -> [64 P, 2 pc, 2 h] f32 for group g."""
    out = np.asarray(bvec, np.float32)[DG * g:][_COLMAP]   # [pc, h, P]
    return np.ascontiguousarray(out.transpose(2, 0, 1))


def make_in_maps(queries, keys, values, Wq, bq, Wk, bk, Wv, Wo):
    def bf(x):
        return np.ascontiguousarray(x).astype(_BF16)

    xq8 = [_pack_x8(queries[b]) for b in range(B)]
    xk8 = [_pack_x8(keys[b]) for b in range(B)]
    xvt = [bf(values[b].T) for b in range(B)]
    in_maps = []
    for core in range(N_CORES):
        b, g = core // 2, core % 2
        sl = slice(g * DG, (g + 1) * DG)
        in_maps.append({
            "xq8": xq8[b],
            "xk8": xk8[b],
            "xv_t": xvt[b],
            "wq8": _pack_w8(Wq, g),
            "wk8": _pack_w8(Wk, g),
            "wv": bf(Wv[:, sl]),
            "wo": bf(Wo[sl, :]),
            "bq8": _pack_b8(bq, g),
            "bk8": _pack_b8(bk, g),
        })
    return in_maps


def _reference_fallback(queries, keys, values, mask, Wq, bq, Wk, bk, Wv, bv,
                        Wo, bo):
    def split_heads(x):
        b, t, c = x.shape
        return x.reshape(b, t, H, c // H).transpose(0, 2, 1, 3)

    q = split_heads(queries @ Wq + bq)
    k = split_heads(keys @ Wk + bk)
    v = split_heads(values @ Wv + bv)
    wei = np.einsum("bhqd,bhkd->bhqk", q, k) * SM_SCALE
    wei = wei + (-1e9) * mask
    wei = wei - wei.max(axis=-1, keepdims=True)
    wei = np.exp(wei)
    wei = wei / wei.sum(axis=-1, keepdims=True)
    out = np.einsum("bhqk,bhkd->bhqd", wei, v)
    out = out.transpose(0, 2, 1, 3).reshape(queries.shape[0],
                                            queries.shape[1], D)
    return (out @ Wo + bo).astype(np.float32)


def kernel(queries, keys, values, mask, Wq, bq, Wk, bk, Wv, bv, Wo, bo):
    queries = np.asarray(queries, np.float32)
    keys = np.asarray(keys, np.float32)
    values = np.asarray(values, np.float32)
    Wq, Wk, Wv, Wo = (np.asarray(w, np.float32) for w in (Wq, Wk, Wv, Wo))
    bq, bk, bv, bo = (np.asarray(v_, np.float32) for v_ in (bq, bk, bv, bo))
    mask2d = np.asarray(mask, np.float32).reshape(T, T)
    causal = np.triu(np.ones((T, T), np.float32), k=1)
    if not np.array_equal(mask2d, causal):
        return _reference_fallback(queries, keys, values,
                                   np.asarray(mask, np.float32),
                                   Wq, bq, Wk, bk, Wv, bv, Wo, bo)

    from concourse.bass_utils import run_bass_kernel_spmd

    nc = _get_compiled()
    in_maps = make_in_maps(queries, keys, values, Wq, bq, Wk, bk, Wv, Wo)
    res = run_bass_kernel_spmd(nc, in_maps, list(range(N_CORES)))
    out = np.zeros((B, T, D), np.float32)
    for core in range(N_CORES):
        out[core // 2] += res.results[core]["y"].astype(np.float32)
    out += bo + bv @ Wo   # value bias is separable (softmax rows sum to 1)
    return out


# revision 40
# speedup vs baseline: 1.5384x; 1.1953x over previous
"""MultiHeadAttention Trainium2 kernel (8 NeuronCores).

Reference computation (B=4, T=2048, D=512, H=8, head_dim=64):
    q = split_heads(queries @ Wq + bq); k, v likewise
    wei = softmax(q k^T / sqrt(512) + (-1e9) * mask)   # mask: causal
    out = merge_heads(wei @ v) @ Wo + bo

Sharding: core = 2*b + g  (b in 0..3 batches, g in 0..1 groups of 4 heads).
Each core computes attention for its batch with its 4 heads plus a partial
output projection through its half of Wo's rows. Host sums the two partials
per batch and adds bo + bv @ Wo (the value bias is separable: softmax rows
sum to 1, so V += bv shifts each head's output by exactly bv).

fp8 path (Q/K only): the host packs X^T and Wq/Wk in fp8e4m3 with features
paired for the PE's DoubleRow mode (2 fp8 rows per cycle):
    x8 [2 s][128 p][2 i][T]      feature d = 256 s + 128 i + p
    w8 [2 s][128 p][2 i][2 pc][2 h][64 m]   out feature = fmap(h, m)
    fmap(h, P) = 64*(P//32) + 32*h + P%32   (head hh = P//32)
The Q/K projection runs as DoubleRow matmuls with M=64 halves landing in
PSUM [64, 2, 512], which the DVE bias-adds and casts straight into the
[64 P][2 pc][2 h][T] fp8 layout that the DoubleRow S-matmul consumes
(lhsT = k8[32hh:+32, pair], rhs = q8[32hh:+32, pair], PE row tiles 0/32).
Softmax logits therefore carry ~1% fp8 noise; V/PV/Wo stay bf16.

ScalarE runs ONLY the softmax exps (the per-core floor). PSUM->SBUF
copies (proj casts, V tiles, output tiles, O^T normalize, denominator row)
are on VectorE; gpsimd handles the k-side input DMA ring, masks and the
denominator DRAM-bounce broadcast. The attention inner loop is software
pipelined (S_{j+1} issued before PV_j) so the in-order PE queue never
waits on the exp it just produced; every emission keeps an EVEN number of
"ps"-tag PSUM allocations so the S tiles stay double-buffered.
"""

import sys

if "/opt/trn_rl_repo" not in sys.path:
    sys.path.insert(0, "/opt/trn_rl_repo")

import numpy as np
import ml_dtypes

B, T, D, H = 4, 2048, 512, 8
HPG = 4                 # heads per group (per core)
HD = 64                 # head dim
DG = HPG * HD           # 256 feature dims per group
N_CORES = 8
SM_SCALE = float(D) ** -0.5   # module scales by full d_k = 512

_BF16 = ml_dtypes.bfloat16
_FP8 = ml_dtypes.float8_e4m3

# out partition P of proj half h -> feature index within the 128-col block
_FMAP = np.array([[64 * (P // 32) + 32 * h + P % 32 for P in range(64)]
                  for h in range(2)])          # [h, P]
# [pc, h, m] -> column within the 256-col group
_COLMAP = np.array([[[128 * pc + _FMAP[h, m] for m in range(64)]
                     for h in range(2)] for pc in range(2)])

_compiled = None


def _build():
    import concourse.bass as bass
    import concourse.bacc as bacc
    import concourse.tile as tile
    import concourse.mybir as mybir

    f32 = mybir.dt.float32
    bf16 = mybir.dt.bfloat16
    fp8 = mybir.dt.float8e4
    Exp = mybir.ActivationFunctionType.Exp
    add = mybir.AluOpType.add
    mult = mybir.AluOpType.mult
    DR = mybir.MatmulPerfMode.DoubleRow

    nc = bacc.Bacc("TRN2", target_bir_lowering=False, debug=False,
                   num_devices=N_CORES)

    xq8 = nc.dram_tensor("xq8", [2, 128, 2, T], fp8,
                         kind="ExternalInput").ap()
    xk8 = nc.dram_tensor("xk8", [2, 128, 2, T], fp8,
                         kind="ExternalInput").ap()
    xv = nc.dram_tensor("xv_t", [D, T], bf16, kind="ExternalInput").ap()
    wq8 = nc.dram_tensor("wq8", [2, 128, 2, 2, 2, 64], fp8,
                         kind="ExternalInput").ap()
    wk8 = nc.dram_tensor("wk8", [2, 128, 2, 2, 2, 64], fp8,
                         kind="ExternalInput").ap()
    wv = nc.dram_tensor("wv", [D, DG], bf16, kind="ExternalInput").ap()
    wo = nc.dram_tensor("wo", [DG, D], bf16, kind="ExternalInput").ap()
    bq8 = nc.dram_tensor("bq8", [64, 2, 2], f32, kind="ExternalInput").ap()
    bk8 = nc.dram_tensor("bk8", [64, 2, 2], f32, kind="ExternalInput").ap()
    y = nc.dram_tensor("y", [T, D], bf16, kind="ExternalOutput").ap()

    NT = T // 512        # 4 t/q chunks of 512
    NB = T // 128        # 16 t/k blocks of 128
    ND = D // 128        # 4 contraction chunks over D

    with tile.TileContext(nc) as tc:
        with (
            tc.tile_pool(name="const", bufs=1) as const,
            tc.tile_pool(name="pt", bufs=10) as ppool,
            tc.tile_pool(name="rc", bufs=4) as rcpool,
            tc.tile_pool(name="ysb", bufs=6) as ypool,
            tc.tile_pool(name="psA", bufs=3, space="PSUM") as psA,
            tc.tile_pool(name="psO", bufs=1, space="PSUM") as psO,
            tc.tile_pool(name="dscratch", bufs=2, space="DRAM") as dpool,
        ):
            # ---- input DMAs: sync ring for q/v side, gpsimd for k side --
            bq_sb = const.tile([64, 2, 2], f32, tag="bq")
            nc.sync.dma_start(out=bq_sb[:], in_=bq8)
            bk_sb = const.tile([64, 2, 2], f32, tag="bk")
            nc.sync.dma_start(out=bk_sb[:], in_=bk8)
            wq_sb = const.tile([128, 2, 2, 2, 2, 64], fp8, tag="wq")
            nc.sync.dma_start(
                out=wq_sb[:], in_=wq8.rearrange("s p i c h m -> p s i c h m"))
            wk_sb = const.tile([128, 2, 2, 2, 2, 64], fp8, tag="wk")
            nc.gpsimd.dma_start(
                out=wk_sb[:], in_=wk8.rearrange("s p i c h m -> p s i c h m"))

            xq_sb = const.tile([128, 2, 2, T], fp8, tag="xq")
            xk_sb = const.tile([128, 2, 2, T], fp8, tag="xk")
            xv_sb = [const.tile([128, T], bf16, tag=f"xv{dc}",
                                name=f"xv{dc}") for dc in range(ND)]
            wv_sb = const.tile([128, ND, DG], bf16, tag="wv")
            wo_sb = const.tile([128, 2, D], bf16, tag="wo")

            def load_x8(sb, dram, lo, hi, eng):
                s = slice(lo, hi)
                for sc in range(2):
                    eng.dma_start(out=sb[:, sc, :, s],
                                  in_=dram[sc, :, :, s])

            # chunk 0 of q/k first: it alone gates the first S block
            load_x8(xq_sb, xq8, 0, 512, nc.sync)
            load_x8(xk_sb, xk8, 0, 512, nc.gpsimd)
            nc.sync.dma_start(
                out=wv_sb[:], in_=wv.rearrange("(c p) m -> p c m", p=128))
            for dc in range(ND):
                nc.sync.dma_start(out=xv_sb[dc][:, 0:1024],
                                  in_=xv[128 * dc:128 * (dc + 1), 0:1024])
            load_x8(xq_sb, xq8, 512, 1024, nc.sync)
            load_x8(xk_sb, xk8, 512, 1024, nc.gpsimd)
            load_x8(xq_sb, xq8, 1024, 2048, nc.sync)
            load_x8(xk_sb, xk8, 1024, 2048, nc.gpsimd)
            for dc in range(ND):
                nc.sync.dma_start(out=xv_sb[dc][:, 1024:2048],
                                  in_=xv[128 * dc:128 * (dc + 1), 1024:2048])
            nc.sync.dma_start(out=wo_sb[:],
                              in_=wo.rearrange("(c p) n -> p c n", p=128))

            # tiny PE warm-up while the DMAs land (a big block would delay
            # the first projection: the PE queue is in-order).
            # NOTE every emission keeps an EVEN number of "ps"-tag psum
            # allocations so the attention S tiles stay double-buffered.
            warm = const.tile([128, 512], bf16, tag="warm")
            nc.vector.memset(warm[:], 0.0)
            for _ in range(2):
                wps = psA.tile([128, 1024], f32, tag="ps", name="wps")
                for _ in range(4):
                    nc.tensor.matmul(wps[:, :512], lhsT=warm[:, :128],
                                     rhs=warm[:], start=True, stop=True)

            # causal masking runs on the PE: the diagonal S window gets
            # ident^T @ negtri accumulated into it (adds -1e5 where q < k,
            # so the exp underflows to exactly 0 and PV needs no mask).
            ident = const.tile([128, 128], bf16, tag="ident")
            nc.gpsimd.memset(ident[:], 1.0)
            nc.gpsimd.affine_select(
                out=ident[:], in_=ident[:],
                compare_op=mybir.AluOpType.is_ge, fill=0.0,
                base=0, pattern=[[1, 128]], channel_multiplier=-1)
            nc.gpsimd.affine_select(
                out=ident[:], in_=ident[:],
                compare_op=mybir.AluOpType.is_ge, fill=0.0,
                base=0, pattern=[[-1, 128]], channel_multiplier=1)
            negtri = const.tile([128, 128], bf16, tag="negtri")
            nc.gpsimd.memset(negtri[:], 0.0)
            nc.gpsimd.affine_select(
                out=negtri[:], in_=negtri[:],
                compare_op=mybir.AluOpType.is_ge, fill=-1e5,
                base=0, pattern=[[1, 128]], channel_multiplier=-1)
            # 0/1 lower-triangular for the DVE-side mask (even diagonals)
            tri = const.tile([128, 128], bf16, tag="tri")
            nc.gpsimd.memset(tri[:], 1.0)
            nc.gpsimd.affine_select(
                out=tri[:], in_=tri[:],
                compare_op=mybir.AluOpType.is_ge, fill=0.0,
                base=0, pattern=[[1, 128]], channel_multiplier=-1)

            q8 = const.tile([64, 2, 2, T], fp8, tag="q8")
            k8 = const.tile([64, 2, 2, T], fp8, tag="k8")
            vA = const.tile([128, NB, HPG, HD + 1], bf16, tag="vA")
            nc.gpsimd.memset(vA[:, :, :, HD:HD + 1], 1.0)
            oTn = const.tile([128, 2, T], bf16, tag="oTn")

            def proj_steps(dst8, w_sb, x_sb, b_sb, pc, c):
                # one 512-wide t-chunk of Q^T or K^T for head pair pc,
                # DoubleRow fp8, M=64 halves -> PSUM [64, 2, 512].
                # 2 micro-steps so the in-order PE queue never stalls the
                # exp feed for long.
                box = {}

                def mms():
                    ps = box["ps"] = psA.tile([64, 2, 512], f32,
                                              tag="ps", name="ps")
                    for h in range(2):
                        for s in range(2):
                            nc.tensor.matmul(
                                ps[:, h, :],
                                lhsT=w_sb[:, s, :, pc, h, :],
                                rhs=x_sb[:, s, :, 512 * c:512 * (c + 1)],
                                start=(s == 0), stop=(s == 1),
                                perf_mode=DR)

                def stt():
                    # bias-add + fp8 cast on VectorE (ScalarE stays
                    # exp-only)
                    nc.vector.scalar_tensor_tensor(
                        dst8[:, pc, :, 512 * c:512 * (c + 1)],
                        box["ps"][:], 1.0,
                        b_sb[:, pc, :, None].to_broadcast((64, 2, 512)),
                        mult, add)

                return [mms, stt]

            def proj_v_steps(tb):
                # two adjacent t-blocks per psum tile, 3 micro-steps
                box = {}

                def mm(s):
                    if s == 0:
                        box["ps"] = psA.tile([128, 2, DG], f32, tag="ps",
                                             name="vps")
                    for dc in range(ND):
                        nc.tensor.matmul(
                            box["ps"][:, s, :],
                            lhsT=xv_sb[dc][:, 128 * (tb + s):
                                           128 * (tb + s + 1)],
                            rhs=wv_sb[:, dc, :],
                            start=(dc == 0), stop=(dc == ND - 1))

                def copy():
                    nc.vector.tensor_copy(
                        vA[:, tb:tb + 2, :, 0:HD],
                        box["ps"].rearrange("p s (h d) -> p s h d",
                                            h=HPG))

                return [lambda: mm(0), lambda: mm(1), copy]

            def out_proj_steps(tb):
                # two adjacent t-blocks per psum tile, 2 micro-steps
                box = {}

                def mms():
                    yp = box["yp"] = psA.tile([128, 2, D], f32, tag="ps",
                                              name="yp")
                    for s in range(2):
                        for pair in range(2):
                            nc.tensor.matmul(
                                yp[:, s, :],
                                lhsT=oTn[:, pair, 128 * (tb + s):
                                         128 * (tb + s + 1)],
                                rhs=wo_sb[:, pair, :],
                                start=(pair == 0), stop=(pair == 1))

                def out():
                    ysb = ypool.tile([128, 2, D], bf16, tag="ysb")
                    nc.vector.tensor_copy(ysb[:], box["yp"][:])
                    nc.sync.dma_start(
                        out=y[128 * tb:128 * (tb + 2), :].rearrange(
                            "(s p) d -> p s d", p=128),
                        in_=ysb[:])

                return [mms, out]

            def attention(pair, qc, inject=(), lag=2):
                # heads 2*pair + hh, hh in {0,1}; PE row tiles 32hh (S).
                # Software pipelined: PV_j is issued `lag` S-blocks after
                # S_j, so the in-order PE queue never stalls the exp feed
                # on a PV whose inputs (vA, pT) are late. inject =
                # [(min_j, fn)]: fn is emitted once j >= min_j (work whose
                # inputs are only ready later never head-of-line blocks).
                inject = list(inject)
                q0 = 512 * qc
                jmax = 4 * qc + 4
                oT = psO.tile([HD + 1, 2, 512], f32, tag="oT", name="oT")
                pts, avs = {}, {}
                for j in range(jmax + lag):
                    if j < jmax:
                        a = avs[j] = max(0, 128 * j - q0)
                        diag = 128 * j >= q0
                        pe_mask = diag and (j % 2 == 1)
                        sT = psA.tile([128, 2, 512], f32, tag="ps",
                                      name="sT")
                        for hh in range(2):
                            o = 32 * hh
                            nc.tensor.matmul(
                                sT[:, hh, a:],
                                lhsT=k8[o:o + 32, pair, :,
                                        128 * j:128 * (j + 1)],
                                rhs=q8[o:o + 32, pair, :, q0 + a:q0 + 512],
                                start=True, stop=not pe_mask,
                                perf_mode=DR,
                                skip_group_check=pe_mask)
                        if pe_mask:  # add -1e5 above the diagonal (PE)
                            for hh in range(2):
                                nc.tensor.matmul(
                                    sT[:, hh, a:a + 128],
                                    lhsT=ident[:],
                                    rhs=negtri[:],
                                    start=False, stop=True,
                                    skip_group_check=True)
                    if j >= lag:
                        jv = j - lag
                        ap = avs.pop(jv)
                        pT = pts.pop(jv)
                        for hh in range(2):
                            nc.tensor.matmul(
                                oT[:, hh, ap:],
                                lhsT=vA[:, jv, 2 * pair + hh, :],
                                rhs=pT[:, hh, ap:],
                                start=(jv == 0), stop=(jv == jmax - 1),
                                skip_group_check=True)
                    if j < jmax:
                        pT = pts[j] = ppool.tile([128, 2, 512], bf16,
                                                 tag="pt", name="pT")
                        nc.scalar.activation(pT[:, :, a:], sT[:, :, a:],
                                             Exp, scale=SM_SCALE)
                        if diag and not pe_mask:  # DVE-side mask
                            nc.vector.tensor_tensor(
                                pT[:, :, a:a + 128], pT[:, :, a:a + 128],
                                tri[:, None, :].to_broadcast((128, 2, 128)),
                                mult)
                    while inject and j >= inject[0][0]:
                        inject.pop(0)[1]()
                for _, fn in inject:
                    fn()
                # stage O^T to SBUF (frees the single psO buffer for the
                # next chunk; row 64 = denominators, recip needs SBUF)
                osb = rcpool.tile([HD + 1, 2, 512], f32, tag="osb")
                nc.vector.tensor_copy(osb[:], oT[:])
                # plain copy to partition 0 first: the custom-DVE recip
                # mishandles nonzero base partitions on hardware
                srow = rcpool.tile([1, 2, 512], f32, tag="srow")
                nc.vector.tensor_copy(srow[:], osb[HD:HD + 1, :, :])
                nc.vector.reciprocal_approx_fast(srow[:], srow[:])
                rc = rcpool.tile([64, 2, 512], f32, tag="rc")
                nc.gpsimd.partition_broadcast(rc[:], srow[:], channels=64)
                for hh in range(2):
                    nc.vector.tensor_tensor(
                        oTn[64 * hh:64 * hh + 64, pair, q0:q0 + 512],
                        osb[0:HD, hh, :], rc[:, hh, :], mult)

            def run(steps):
                for s in steps:
                    s()

            def pq(pc, c):
                return proj_steps(q8, wq_sb, xq_sb, bq_sb, pc, c)

            def pk(pc, c):
                return proj_steps(k8, wk_sb, xk_sb, bk_sb, pc, c)

            def slot(start, *step_lists):
                out = []
                i = start
                for steps in step_lists:
                    for s in steps:
                        out.append((i, s))
                        i += 1
                return out

            # ---- schedule ------------------------------------------------
            run(pq(0, 0))
            run(pk(0, 0))
            attention(0, 0, slot(0, pq(0, 1), pk(0, 1)) +
                      [(3, lambda: run(proj_v_steps(0) +
                                       proj_v_steps(2)))],
                      lag=4)
            attention(0, 1, slot(0, proj_v_steps(4), proj_v_steps(6),
                                 pq(0, 2), pk(0, 2)))
            attention(0, 2, slot(0, proj_v_steps(8), proj_v_steps(10),
                                 pq(0, 3), pk(0, 3),
                                 proj_v_steps(12), proj_v_steps(14)))
            attention(0, 3, slot(0, pq(1, 0), pk(1, 0)))
            attention(1, 0, slot(0, pq(1, 1), pk(1, 1)))
            attention(1, 1, slot(0, pq(1, 2), pk(1, 2)) +
                      slot(5, out_proj_steps(0), out_proj_steps(2)))
            attention(1, 2, slot(0, pq(1, 3), pk(1, 3)) +
                      slot(5, out_proj_steps(4), out_proj_steps(6)))
            attention(1, 3, slot(5, out_proj_steps(8), out_proj_steps(10)))
            run(out_proj_steps(12))
            run(out_proj_steps(14))

    nc.compile()
    return nc


def _get_compiled():
    global _compiled
    if _compiled is None:
        _compiled = _build()
    return _compiled


def _pack_x8(x):
    """X [T, 512] -> [2 s, 128 p, 2 i, T] fp8 (feature d = 256s+128i+p)."""
    xt = np.ascontiguousarray(x.T).astype(_FP8)        # [512, T]
    return np.ascontiguousarray(
        xt.reshape(2, 2, 128, T).transpose(0, 2, 1, 3))


def _pack_w8(w, g):
    """W [512, 512] -> [2 s, 128 p, 2 i, 2 pc, 2 h, 64 m] fp8 for group g."""
    wg = w[:, DG * g:DG * (g + 1)].astype(_FP8)        # [512, 256]
    wr = wg.reshape(2, 2, 128, DG)                     # [s, i, p, col]
    out = wr[:, :, :, _COLMAP]                         # [s, i, p, pc, h, m]
    return np.ascontiguousarray(out.transpose(0, 2, 1, 3, 4, 5))


def _pack_b8(bvec, g):
    """b [512] -> [64 P, 2 pc, 2 h] f32 for group g."""
    out = np.asarray(bvec, np.float32)[DG * g:][_COLMAP]   # [pc, h, P]
    return np.ascontiguousarray(out.transpose(2, 0, 1))


def make_in_maps(queries, keys, values, Wq, bq, Wk, bk, Wv, Wo):
    def bf(x):
        return np.ascontiguousarray(x).astype(_BF16)

    xq8 = [_pack_x8(queries[b]) for b in range(B)]
    xk8 = [_pack_x8(keys[b]) for b in range(B)]
    xvt = [bf(values[b].T) for b in range(B)]
    in_maps = []
    for core in range(N_CORES):
        b, g = core // 2, core % 2
        sl = slice(g * DG, (g + 1) * DG)
        in_maps.append({
            "xq8": xq8[b],
            "xk8": xk8[b],
            "xv_t": xvt[b],
            "wq8": _pack_w8(Wq, g),
            "wk8": _pack_w8(Wk, g),
            "wv": bf(Wv[:, sl]),
            "wo": bf(Wo[sl, :]),
            "bq8": _pack_b8(bq, g),
            "bk8": _pack_b8(bk, g),
        })
    return in_maps


def _reference_fallback(queries, keys, values, mask, Wq, bq, Wk, bk, Wv, bv,
                        Wo, bo):
    def split_heads(x):
        b, t, c = x.shape
        return x.reshape(b, t, H, c // H).transpose(0, 2, 1, 3)

    q = split_heads(queries @ Wq + bq)
    k = split_heads(keys @ Wk + bk)
    v = split_heads(values @ Wv + bv)
    wei = np.einsum("bhqd,bhkd->bhqk", q, k) * SM_SCALE
    wei = wei + (-1e9) * mask
    wei = wei - wei.max(axis=-1, keepdims=True)
    wei = np.exp(wei)
    wei = wei / wei.sum(axis=-1, keepdims=True)
    out = np.einsum("bhqk,bhkd->bhqd", wei, v)
    out = out.transpose(0, 2, 1, 3).reshape(queries.shape[0],
                                            queries.shape[1], D)
    return (out @ Wo + bo).astype(np.float32)


def kernel(queries, keys, values, mask, Wq, bq, Wk, bk, Wv, bv, Wo, bo):
    queries = np.asarray(queries, np.float32)
    keys = np.asarray(keys, np.float32)
    values = np.asarray(values, np.float32)
    Wq, Wk, Wv, Wo = (np.asarray(w, np.float32) for w in (Wq, Wk, Wv, Wo))
    bq, bk, bv, bo = (np.asarray(v_, np.float32) for v_ in (bq, bk, bv, bo))
    mask2d = np.asarray(mask, np.float32).reshape(T, T)
    causal = np.triu(np.ones((T, T), np.float32), k=1)
    if not np.array_equal(mask2d, causal):
        return _reference_fallback(queries, keys, values,
                                   np.asarray(mask, np.float32),
                                   Wq, bq, Wk, bk, Wv, bv, Wo, bo)

    from concourse.bass_utils import run_bass_kernel_spmd

    nc = _get_compiled()
    in_maps = make_in_maps(queries, keys, values, Wq, bq, Wk, bk, Wv, Wo)
    res = run_bass_kernel_spmd(nc, in_maps, list(range(N_CORES)))
    out = np.zeros((B, T, D), np.float32)
    for core in range(N_CORES):
        out[core // 2] += res.results[core]["y"].astype(np.float32)
    out += bo + bv @ Wo   # value bias is separable (softmax rows sum to 1)
    return out
